# revision 54
# baseline (speedup 1.0000x reference)
"""Block-sparse linear kernel for Trainium2 (8 NeuronCores, SPMD data-parallel).

Computes y = x @ (W * mask) + bias for
    x    [8, 1024, 4096] f32
    W    [4096, 4096]    f32
    mask [4096, 4096]    int32 (32x32-block structured, ~25% block density)
    bias [4096]          f32
    y    [8, 1024, 4096] f32

Strategy
--------
- Data parallel: core c computes rows [1024c, 1024(c+1)) of the flattened
  [8192, 4096] activation (i.e. batch element c).
- The trn2 PE array runs in 64x32 tiling mode (8 concurrent sub-array
  positions).  HW-measured: a tiled LDWEIGHTS+MATMUL stream sustains a
  FLAT ~31-32ns per MATMUL at 8 positions regardless of N (128/256/512),
  LDW count, or 16-way tiling — so the only lever that matters is the
  MATMUL COUNT.  The kernel therefore packs present 32x32 blocks into as
  few 64x32 panels as possible:
    * TWO copies of x live in SBUF under different block-row pairings
      (max-weight matching A; complementary matching B targeting the
      columns A leaves single).  Each block-column covers its present
      blocks by a per-column min edge cover over the union of the two
      pairings (panels per m-slice drop ~13%; every present block is
      covered exactly once, unused panel slots are zero-filled).
    * Block-columns are assigned to (supertile, col-slot) by a local
      search balancing the 8 per-supertile position queues (round-robin
      drain => imbalance = tail rounds with idle positions).
- Each weight panel is loaded into the PE array ONCE and used for BOTH
  512-column m-slices: bass's legalizer emits an LDWEIGHTS per matmul and
  a post-pass deletes the duplicate (the array keeps the stationary
  operand across matmuls — validated on HW).
- Concurrent DMA traffic measurably degrades the PE issue rate, so DMA is
  shaped: x tiles ride 4KB-line grouped DMAs (descriptor-rate-bound
  queues), copy A's m0 halves land first and a m0-only chunk-major ramp
  over copy-A-only supertiles tracks their arrival, weights prefetch
  W_PRE supertiles ahead (strip r2=0 on the SWDGE gpsimd queue), and the
  evacuation is one [128,1024] bf16 DMA per supertile.
- The two 64-row groups write disjoint PSUM banks; VectorE reduces the 2
  partial banks straight into a bf16 tile (halves evac DMA bytes; the
  harness gate is 2e-2 rel err, bf16 output adds ~2e-3).  Host assembles
  the final fp32 output, un-permuting the supertile column assignment.
"""

import numpy as np
import ml_dtypes

B, S, IN_F, OUT_F = 8, 1024, 4096, 4096
BS = 32                      # sparsity block size
GI, GJ = IN_F // BS, OUT_F // BS
GP = GI // 2                 # vertical super-rows (64 rows each)
N_CORES = 8
M_CORE = (B * S) // N_CORES  # rows of x per core (1024)
MSL = 512                    # m-slice width (one PSUM bank of fp32)
N_MSL = M_CORE // MSL        # 2
JCOLS = 4                    # output block-columns per supertile (4*32 = 128 partitions)
N_J = GJ // JCOLS            # 32 output supertiles
N_T = IN_F // 128            # 32 x tiles of 128 rows

BF16 = ml_dtypes.bfloat16


def _ensure_ntff_hook():
    """Best-effort: make trace=True work under axon when the image's antenv
    lacks axon_hooks.  Harmless if it fails — tracing is skipped, results
    are still correct."""
    import sys, types
    try:
        import antenv  # noqa
    except ImportError:
        return
    try:
        from antenv.axon_hooks import get_axon_ntff_profile_hook
        if get_axon_ntff_profile_hook() is not None:
            return
        mod = sys.modules["antenv.axon_hooks"]
    except ImportError:
        mod = types.ModuleType("antenv.axon_hooks")
        mod._hook = None
        def set_axon_ntff_profile_hook(h, _m=mod):
            _m._hook = h
        def get_axon_ntff_profile_hook(_m=mod):
            return _m._hook
        mod.set_axon_ntff_profile_hook = set_axon_ntff_profile_hook
        mod.get_axon_ntff_profile_hook = get_axon_ntff_profile_hook
        sys.modules["antenv.axon_hooks"] = mod
        import antenv as _a
        _a.axon_hooks = mod
    try:
        from trn_agent_boot.trn_boot import _ntff_profile_via_ctypes
        mod.set_axon_ntff_profile_hook(
            _ntff_profile_via_ctypes("/opt/axon/libaxon_pjrt.so")
        )
    except Exception:
        pass


def _pair_permutation(nzb):
    """Order block-rows so vertically-paired rows co-occur in many columns.

    Greedy max-weight matching on C[a,b] = #columns where blocks a and b are
    both present; each matched pair becomes one 64-row super-row, so high
    weight = fewer half-empty 64x32 panels = fewer matmuls.
    """
    C = nzb.astype(np.int32) @ nzb.astype(np.int32).T
    pairs = []
    try:
        import networkx as nx
        G = nx.Graph()
        for a in range(GI):
            for b in range(a + 1, GI):
                G.add_edge(a, b, weight=int(C[a, b]))
        pairs = [
            (int(min(a, b)), int(max(a, b)))
            for a, b in nx.max_weight_matching(G, maxcardinality=True)
        ]
    except Exception:
        pass
    if len(pairs) != GI // 2:
        pairs = []
        iu = np.triu_indices(GI, k=1)
        order = np.argsort(C[iu])[::-1]
        used = np.zeros(GI, dtype=bool)
        for idx in order:
            a, b = iu[0][idx], iu[1][idx]
            if not used[a] and not used[b]:
                used[a] = used[b] = True
                pairs.append((int(a), int(b)))
                if len(pairs) == GI // 2:
                    break
    perm = []
    for a, b in pairs:
        perm.extend((a, b))
    for a in range(GI):      # safety for odd leftovers
        if a not in perm:
            perm.append(a)
    return np.asarray(perm)


def _second_matching(nzb, permA):
    """Complementary pairing: match rows that copy A leaves as singletons in
    many columns."""
    partnerA = {}
    for p in range(GI // 2):
        a, b = int(permA[2 * p]), int(permA[2 * p + 1])
        partnerA[a] = b
        partnerA[b] = a
    presentA_partner = np.zeros((GI, GJ), bool)
    for r in range(GI):
        presentA_partner[r] = nzb[partnerA[r]]
    waste = nzb & ~presentA_partner
    Wb = waste.astype(np.int32) @ waste.astype(np.int32).T
    for a, b in partnerA.items():
        Wb[a, b] = -1000
    pairs = []
    try:
        import networkx as nx
        G = nx.Graph()
        for a in range(GI):
            for b in range(a + 1, GI):
                G.add_edge(a, b, weight=int(Wb[a, b]) + 1000)
        pairs = [(min(a, b), max(a, b))
                 for a, b in nx.max_weight_matching(G, maxcardinality=True)]
    except Exception:
        pass
    if len(pairs) != GI // 2:
        pairs = []
        iu = np.triu_indices(GI, k=1)
        order = np.argsort(Wb[iu])[::-1]
        used = np.zeros(GI, dtype=bool)
        for idx in order:
            a, b = int(iu[0][idx]), int(iu[1][idx])
            if not used[a] and not used[b]:
                used[a] = used[b] = True
                pairs.append((a, b))
                if len(pairs) == GI // 2:
                    break
    permB = []
    for a, b in pairs:
        permB.extend((a, b))
    for a in range(GI):
        if a not in permB:
            permB.append(a)
    return np.asarray(permB)


def _cover_columns(nzb, perms, a_only_cols):
    """Per column, cover the present rows with pair-panels from the given
    copies (min edge cover over the union of the pair matchings).

    nzb: [GI, GJ] presence in ORIGINAL row indices.
    perms: [permA, permB].
    a_only_cols: set of j restricted to copy A (their x copy-B data would
    not be resident early enough).

    Returns covers[j] = list of (copy, p, use0, use1) — pair index p of
    that copy; use0/use1 say whether the top/bottom 32-row slot carries
    real weights (False slots are zero-filled).  Every present block is
    covered exactly once.
    """
    pair_of = []       # per copy: row -> (p, slot)
    for perm in perms:
        m = {}
        for p in range(GI // 2):
            m[int(perm[2 * p])] = (p, 0)
            m[int(perm[2 * p + 1])] = (p, 1)
        pair_of.append(m)

    covers = []
    for j in range(GJ):
        R = set(np.where(nzb[:, j])[0].tolist())
        ncopies = 1 if j in a_only_cols else len(perms)
        # full edges: both endpoints present, edge = a copy's pair
        adj = {r: [] for r in R}
        for C in range(ncopies):
            perm = perms[C]
            for p in range(GI // 2):
                a, b = int(perm[2 * p]), int(perm[2 * p + 1])
                if a in R and b in R:
                    adj[a].append((b, C, p))
                    adj[b].append((a, C, p))
        matched = {}
        visited = set()
        for start in sorted(R):
            if start in visited:
                continue
            comp = set()
            stack = [start]
            while stack:
                v = stack.pop()
                if v in comp:
                    continue
                comp.add(v)
                for w, _C, _p in adj[v]:
                    if w not in comp:
                        stack.append(w)
            visited |= comp
            deg = {v: len(adj[v]) for v in comp}
            ends = sorted(v for v in comp if deg[v] <= 1)
            cur = ends[0] if ends else min(comp)
            prev = None
            order = []
            seen = set()
            while cur is not None and cur not in seen:
                order.append(cur)
                seen.add(cur)
                nxt = None
                for w, _C, _p in adj[cur]:
                    if w != prev and w not in seen:
                        nxt = w
                        break
                prev, cur = cur, nxt
            i = 0
            while i < len(order):
                if i + 1 < len(order):
                    e = next(((C, p) for w, C, p in adj[order[i]]
                              if w == order[i + 1]), None)
                    if e is not None:
                        matched[order[i]] = (e, order[i + 1])
                        matched[order[i + 1]] = (e, order[i])
                        i += 2
                        continue
                i += 1
        panels = {}
        for r in sorted(R):
            if r in matched:
                (C, p), partner = matched[r]
                slot = pair_of[C][r][1]
                key = (C, p)
                u = panels.setdefault(key, [False, False])
                u[slot] = True
            else:
                # singleton: use copy A's pair (or B if j allows and it
                # balances parity — keep simple: A)
                C = 0
                p, slot = pair_of[C][r]
                u = panels.setdefault((C, p + 10000 * (slot + 1)), [False, False])
                # distinct key per slot so two unrelated singletons that
                # happen to share a pair don't merge incorrectly — unless
                # they are the two slots of the same pair, which IS a valid
                # merge; handle below
                u[slot] = True
        # merge singleton keys back: (C, p+10000*(s+1)) pairs of same p
        merged = {}
        for (C, pk), u in panels.items():
            p = pk % 10000
            key = (C, p)
            mu = merged.setdefault(key, [False, False])
            mu[0] |= u[0]
            mu[1] |= u[1]
        covers.append([(C, p, u[0], u[1]) for (C, p), u in sorted(merged.items())])
    return covers


def _balance_jsel(covers, early_cols):
    """Assign block-columns j to (supertile J, col-slot c) so the 8 per-
    supertile position queues are balanced (the schedule drains round-robin,
    so spread = tail rounds with idle positions).  Columns in early_cols are
    pinned to the ramp supertiles (their covers use only copy A, whose x
    lands first).

    Returns jsel [N_J][4].
    """
    import random
    n = np.zeros((GJ, 2), np.int64)
    for j in range(GJ):
        for C, p, u0, u1 in covers[j]:
            n[j, p % 2] += 1

    def balance(cols, ngroups, iters):
        def loss(groups):
            tot = 0
            for g in groups:
                for r2 in range(2):
                    lens = [max(int(n[j, r2]), 1) for j in g]
                    tot += 4 * max(lens) - sum(lens)
            return tot

        cols = sorted(cols, key=lambda j: -int(n[j].sum()))
        groups = [cols[4 * J : 4 * J + 4] for J in range(ngroups)]
        cur = loss(groups)
        rng = random.Random(0)
        for _ in range(iters):
            a, b = rng.randrange(ngroups), rng.randrange(ngroups)
            if a == b:
                continue
            ia, ib = rng.randrange(4), rng.randrange(4)
            groups[a][ia], groups[b][ib] = groups[b][ib], groups[a][ia]
            nl = loss(groups)
            if nl <= cur:
                cur = nl
            else:
                groups[a][ia], groups[b][ib] = groups[b][ib], groups[a][ia]
        return groups

    n_early_groups = len(early_cols) // 4
    rest = [j for j in range(GJ) if j not in set(early_cols)]
    return (balance(list(early_cols), n_early_groups, 20000)
            + balance(rest, N_J - n_early_groups, 60000))


def _plan(covers, jsel):
    """Per-supertile weight storage layout and position queues (64x32
    pairing over two x copies).

    covers: per column j, list of (copy, p, use0, use1) pair-panels.
    jsel: [N_J][4] — block-column assigned to (J, c).

    Returns (plan, strip_cols):
      plan[J] = {
        'chunks': {r2: (src_col_base, n_cells)},     # weight DMA per strip
        'cells':  {r2: [(C, p, use0, use1, j), ...]},  # storage order
        'queues': {(r2, c): [(woff_or_None, C, p), ...]},
      }
    r2 = p % 2 (which 64-row half of the copy's x tile the pair occupies).
    woff None => dummy matmul with the zero-weight tile (position had no
    cells but its PSUM region must be initialized for the bank reduce).
    """
    plan = []
    strip_cols = [0, 0]
    for J in range(N_J):
        per_strip = {0: [], 1: []}
        ents = []
        for c in range(4):
            j = jsel[J][c]
            for C, p, u0, u1 in covers[j]:
                ents.append((C, p // 2, p, u0, u1, j, c))
        # x-arrival order: copy A's tiles land before copy B's
        ents.sort(key=lambda e: (e[0], e[1], e[6]))
        for C, lt, p, u0, u1, j, c in ents:
            per_strip[p % 2].append((C, p, u0, u1, j, c))
        chunks = {}
        cells = {}
        queues = {}
        for r2 in range(2):
            lst = per_strip[r2]
            chunks[r2] = (strip_cols[r2], len(lst))
            cells[r2] = [(C, p, u0, u1, j) for C, p, u0, u1, j, _c in lst]
            strip_cols[r2] += len(lst) * BS
            for k, (C, p, u0, u1, j, c) in enumerate(lst):
                queues.setdefault((r2, c), []).append((k * BS, C, p))
        for r2 in range(2):
            for c in range(4):
                if (r2, c) not in queues:
                    queues[(r2, c)] = [(None, 0, r2)]
        plan.append({"chunks": chunks, "cells": cells, "queues": queues})
    return plan, strip_cols


def _dedup_ldweights(nc):
    """Delete InstLdweights whose weights AP + tile position match the most
    recently loaded weights in the same basic block.  The PE array keeps the
    stationary operand across matmuls (validated on HW), so the reload is
    pure overhead on the weight-streaming path.
    """
    ndel = 0
    for f in nc.m.functions:
        for bb in f.blocks:
            insts = bb.instructions
            keep = []
            last = None
            for ins in insts:
                tn = type(ins).__name__
                if tn == 'InstLdweights':
                    k = (str(ins.ins[0]), str(ins.tile_position),
                         str(ins.tile_size), str(ins.perf_mode))
                    if k == last:
                        si = ins.sync_info
                        assert si is None or (
                            len(si.on_wait) == 0 and len(si.on_update) == 0
                        ), f"dup LDW {ins.name} carries sync info"
                        ndel += 1
                        continue
                    last = k
                elif tn == 'InstMatmult' and ins.is_transpose:
                    last = None  # transpose loads identity into the array
                keep.append(ins)
            if len(keep) != len(insts):
                while len(insts):
                    insts.pop()
                for i in keep:
                    insts.append(i)
    return ndel


def _build_program(plan, strip_cols):
    import concourse.bacc as bacc
    import concourse.tile as tile
    import concourse.mybir as mybir

    nc = bacc.Bacc(debug=False)
    bf16, f32 = mybir.dt.bfloat16, mybir.dt.float32

    XG = 4                       # x tiles per DMA group (4KB DRAM lines)
    N_G = (2 * N_T) // XG        # two x copies: A = groups 0..7, B = 8..15
    xt_d = nc.declare_dram_parameter(
        "xt", [N_G, 128, N_MSL, XG * MSL], bf16, isOutput=False
    )
    w_d = {}
    for r2 in range(2):
        if strip_cols[r2] > 0:
            w_d[r2] = nc.declare_dram_parameter(
                f"w{r2}", [2 * BS, strip_cols[r2]], bf16, isOutput=False
            )
    out_d = nc.declare_dram_parameter("out", [OUT_F, M_CORE], bf16, isOutput=True)

    # Largest per-(J, strip) weight chunk, in columns (>= BS for the tile alloc).
    lmax = BS
    for p in plan:
        for r2 in range(2):
            lmax = max(lmax, p["chunks"][r2][1] * BS)

    W_PRE = 8   # weight prefetch depth in supertiles
    N_GEN = 4   # ramp supertiles (m0-only pass, then m1 pass)

    with tile.TileContext(nc) as tc:
        with (
            tc.tile_pool(name="xp", bufs=1) as xp,
            tc.tile_pool(name="zp", bufs=1) as zp,
            tc.tile_pool(name="wp", bufs=W_PRE + 2) as wp,
            tc.tile_pool(name="ep", bufs=10) as ep,
            tc.tile_pool(name="pp", bufs=4, space="PSUM") as pp,
        ):
            # Queue assignment: x and evacs ride the HWDGE queues (sync,
            # scalar); weight strip r2=0 rides the SWDGE gpsimd queue (its
            # latency is hidden by the W_PRE prefetch depth), strip r2=1
            # alternates the HWDGE queues.
            def load_w(J):
                wt = wp.tile([128, lmax], bf16, tag="wt")
                for r2 in range(2):
                    base, ncell = plan[J]["chunks"][r2]
                    if ncell:
                        eng = (
                            nc.gpsimd if r2 == 0
                            else (nc.sync if J % 2 == 0 else nc.scalar)
                        )
                        eng.dma_start(
                            wt[64 * r2 : 64 * r2 + 64, : ncell * BS],
                            w_d[r2][:, base : base + ncell * BS],
                        )
                return wt

            Xg = {}

            def load_xg(g, m):
                # one DMA per (4-tile group, m-slice): 4KB contiguous per
                # partition on both the DRAM and SBUF side, so the
                # descriptor-rate-bound DMA queues move 4KB per descriptor;
                # m0 chunks land first — they are all the m0-only ramp needs
                if g not in Xg:
                    xg = xp.tile([128, N_MSL, XG, MSL], bf16, tag=f"xg{g}")
                    Xg[g] = xg
                (nc.sync if g % 2 == 0 else nc.scalar).dma_start(
                    Xg[g][:, m], xt_d[g][:, m]
                )

            # DMA emission order: ramp weights interleaved with copy A's m0
            # groups (the m0-only ramp input), then copy B m0 alternating
            # with copy A m1, then copy B m1; remaining weights are emitted
            # inside the J loop, W_PRE supertiles ahead.
            NGC = N_G // 2           # groups per copy
            wts = {0: load_w(0)}
            load_xg(0, 0)
            wts[1] = load_w(1)
            load_xg(1, 0)
            wts[2] = load_w(2)
            load_xg(2, 0)
            wts[3] = load_w(3)
            for g in range(3, NGC):
                load_xg(g, 0)
            wts[4] = load_w(4)
            wts[5] = load_w(5)
            for g in range(NGC):
                load_xg(NGC + g, 0)   # copy B m0
                load_xg(g, 1)         # copy A m1
                if g == 1:
                    wts[6] = load_w(6)
                if g == 3:
                    wts[7] = load_w(7)
            for g in range(NGC):
                load_xg(NGC + g, 1)   # copy B m1
            zw = zp.tile([128, BS], bf16)
            nc.vector.memset(zw[:], 0.0)

            def emit_mm(P, wt, r2, c, woff, C, p, m, start, stop):
                lhsT = (
                    zw[64 * r2 : 64 * r2 + 64, :]
                    if woff is None
                    else wt[64 * r2 : 64 * r2 + 64, woff : woff + BS]
                )
                t = C * N_T + p // 2   # global x tile of this pair
                nc.tensor.matmul(
                    P[32 * c : 32 * c + 32, r2, :],
                    lhsT,
                    Xg[t // XG][64 * r2 : 64 * r2 + 64, m, t % XG, :],
                    start=start,
                    stop=stop,
                    tile_position=(64 * r2, 32 * c),
                )

            def emit_pair(P0, P1, wt, r2, c, woff, C, p, start, stop):
                emit_mm(P0, wt, r2, c, woff, C, p, 0, start, stop)
                emit_mm(P1, wt, r2, c, woff, C, p, 1, start, stop)

            def emit_reduce(P, ov, m):
                with nc.allow_low_precision(
                    reason="bf16 output; harness gate is 2e-2 rel err"
                ):
                    nc.vector.reduce_sum(
                        ov[:, m * MSL : (m + 1) * MSL],
                        P[:].transpose([0, 2, 1]),
                        axis=mybir.AxisListType.X,
                    )

            def emit_out(ov, J):
                # one [128, 1024] DMA per supertile: 2KB DRAM lines.  Early
                # supertiles evacuate on the SWDGE gpsimd queue so the HWDGE
                # queues stay clear for x during the landing-critical window;
                # evac latency is absorbed by the ep pool depth.
                eng = (
                    nc.gpsimd if J < 12
                    else (nc.scalar if J % 2 == 0 else nc.sync)
                )
                eng.dma_start(out_d[128 * J : 128 * (J + 1), :], ov[:])

            POS = [(r2, c) for r2 in range(2) for c in range(4)]

            # Ramp: the first N_GEN supertiles' queues merged chunk-major
            # (all ramp supertiles' panels for x tile t before any of tile
            # t+1) and m0-only first — early compute tracks x-group arrival;
            # the m1 pass over the same supertiles follows.  PSUM: N_GEN
            # single-m groups = all 8 banks.
            # Ramp covers 2*N_GEN supertiles in four m-split phases:
            # [J0-3 m0][J4-7 m0][J0-3 m1][J4-7 m1].  The first two phases
            # consume only copy A's m0 halves (which land first), giving the
            # m1 halves twice as long to arrive before phase 3 needs them.
            GEN0 = list(range(min(2 * N_GEN, N_J)))
            halves = [GEN0[:N_GEN], GEN0[N_GEN:]]
            mergeds = []
            for half in halves:
                merged = []
                for J in half:
                    for (r2, c), q in plan[J]["queues"].items():
                        for k, (woff, C, p) in enumerate(q):
                            t = -1 if woff is None else C * N_T + p // 2
                            merged.append((t, k, J, r2, c, woff, C, p))
                merged.sort(key=lambda e: (e[0], e[1], e[2]))
                first_of = {}
                last_of = {}
                for idx, e in enumerate(merged):
                    key = (e[2], e[3], e[4])
                    first_of.setdefault(key, idx)
                    last_of[key] = idx
                mergeds.append((merged, first_of, last_of))

            OV = {}
            for J in GEN0:
                ovg = ep.tile([128, M_CORE], bf16, tag="ov")
                OV[J] = ovg
            for m in range(N_MSL):
                for hi, half in enumerate(halves):
                    if not half:
                        continue
                    merged, first_of, last_of = mergeds[hi]
                    P_gen = {}
                    for J in half:
                        Pg = pp.tile([128, 2, MSL], f32, tag="P")
                        P_gen[J] = Pg
                    for idx, (t, k, J, r2, c, woff, C, p) in enumerate(merged):
                        key = (J, r2, c)
                        emit_mm(
                            P_gen[J], wts[J], r2, c, woff, C, p, m,
                            first_of[key] == idx, last_of[key] == idx,
                        )
                    for J in half:
                        emit_reduce(P_gen[J], OV[J], m)
                        if m == N_MSL - 1:
                            emit_out(OV[J], J)

            # Main loop: globally pipelined round-robin over the 8 positions
            # with a 2-supertile PSUM window — a position that drains its
            # queue for supertile J immediately starts on J+1, so supertile
            # tails don't idle positions.
            PIPELINE = _build_program.pipeline
            if not PIPELINE:
                next_w = max(wts.keys()) + 1
                for J in range(len(GEN0), N_J):
                    while next_w < min(J + W_PRE, N_J):
                        wts[next_w] = load_w(next_w)
                        next_w += 1
                    wt = wts.pop(J)
                    P0 = pp.tile([128, 2, MSL], f32, tag="P")
                    P1 = pp.tile([128, 2, MSL], f32, tag="P")
                    qlists = [plan[J]["queues"][p] for p in POS]
                    idx = [0] * len(qlists)
                    remaining = sum(len(q) for q in qlists)
                    while remaining:
                        for qi, q in enumerate(qlists):
                            if idx[qi] < len(q):
                                woff, C, p = q[idx[qi]]
                                emit_pair(
                                    P0, P1, wt, POS[qi][0], POS[qi][1],
                                    woff, C, p,
                                    idx[qi] == 0, idx[qi] == len(q) - 1,
                                )
                                idx[qi] += 1
                                remaining -= 1
                    ov = ep.tile([128, M_CORE], bf16, tag="ov")
                    emit_reduce(P0, ov, 0)
                    emit_reduce(P1, ov, 1)
                    emit_out(ov, J)
            else:
                next_w = max(wts.keys()) + 1
                live = []
                pos_at = {}
                qpos = {}          # (J, p) -> next entry index

                def open_J(J):
                    nonlocal next_w
                    while next_w < min(J + W_PRE, N_J):
                        wts[next_w] = load_w(next_w)
                        next_w += 1
                    P0 = pp.tile([128, 2, MSL], f32, tag="P")
                    P1 = pp.tile([128, 2, MSL], f32, tag="P")
                    live.append({"J": J, "P": (P0, P1), "wt": wts.pop(J),
                                 "done": set()})

                def close_J(entry):
                    J = entry["J"]
                    ov = ep.tile([128, M_CORE], bf16, tag="ov")
                    emit_reduce(entry["P"][0], ov, 0)
                    emit_reduce(entry["P"][1], ov, 1)
                    emit_out(ov, J)

                J0 = len(GEN0)
                for p in POS:
                    pos_at[p] = J0
                open_J(J0)
                if J0 + 1 < N_J:
                    open_J(J0 + 1)
                while live:
                    progressed = False
                    for p in POS:
                        Jp = pos_at[p]
                        entry = next((e for e in live if e["J"] == Jp), None)
                        if entry is None:
                            continue
                        q = plan[Jp]["queues"][p]
                        k = qpos.get((Jp, p), 0)
                        if k >= len(q):
                            entry["done"].add(p)
                            if any(e["J"] == Jp + 1 for e in live):
                                pos_at[p] = Jp + 1
                            continue
                        woff, C, pp_ = q[k]
                        emit_pair(
                            entry["P"][0], entry["P"][1], entry["wt"],
                            p[0], p[1], woff, C, pp_,
                            k == 0, k == len(q) - 1,
                        )
                        qpos[(Jp, p)] = k + 1
                        progressed = True
                    head = live[0]
                    if len(head["done"]) == len(POS):
                        close_J(head)
                        live.pop(0)
                        nxt = (live[-1]["J"] + 1) if live else head["J"] + 1
                        if nxt < N_J:
                            open_J(nxt)
                            for p in POS:
                                if pos_at[p] < live[0]["J"]:
                                    pos_at[p] = live[0]["J"]
                    if not progressed:
                        for e in live:
                            for p in POS:
                                if p not in e["done"]:
                                    q = plan[e["J"]]["queues"][p]
                                    if qpos.get((e["J"], p), 0) >= len(q):
                                        e["done"].add(p)

    ndel = _dedup_ldweights(nc)
    nc.compile()
    nc._ldw_dedup_count = ndel
    return nc


_build_program.pipeline = False


_CACHE = {}


def kernel(x, W, bias, mask):
    assert x.shape == (B, S, IN_F) and W.shape == (IN_F, OUT_F)
    _ensure_ntff_hook()
    from concourse.bass_utils import run_bass_kernel_spmd

    # --- host-side input prep -------------------------------------------
    mask_nz = mask != 0
    nzb = np.asarray(mask_nz.reshape(GI, BS, GJ, BS).any(axis=(1, 3)))

    key = nzb.tobytes()
    if key not in _CACHE:
        # alternating optimization of the two pairings: re-complement each
        # against the other and keep the pair with the smallest total cover
        permA = _pair_permutation(nzb)
        permB = _second_matching(nzb, permA)
        best = None
        cand = (permA, permB)
        for _ in range(3):
            size = sum(len(c) for c in _cover_columns(nzb, list(cand), set()))
            if best is None or size < best[0]:
                best = (size, cand)
            cand = (_second_matching(nzb, cand[1]), cand[1])
            size = sum(len(c) for c in _cover_columns(nzb, list(cand), set()))
            if size < best[0]:
                best = (size, cand)
            cand = (cand[0], _second_matching(nzb, cand[0]))
        perms = list(best[1])
        # pick ramp (copy-A-only) columns: those that gain least from copy B
        covA = _cover_columns(nzb, perms, set(range(GJ)))
        covAB = _cover_columns(nzb, perms, set())
        penalty = [len(covA[j]) - len(covAB[j]) for j in range(GJ)]
        N_EARLY = 32
        early = sorted(range(GJ), key=lambda j: (penalty[j], j))[:N_EARLY]
        covers = _cover_columns(nzb, perms, set(early))
        jsel = _balance_jsel(covers, early)
        plan, strip_cols = _plan(covers, jsel)
        nc = _build_program(plan, strip_cols)
        _CACHE[key] = (perms, jsel, plan, strip_cols, nc)
    perms, jsel, plan, strip_cols, nc = _CACHE[key]

    # Masked weights, gathered per row strip in plan storage order.  Unused
    # panel slots (use flag False) are zero-filled: their block may be
    # present in the mask but is covered by another panel.
    Wm = np.where(mask_nz, W, np.float32(0)).astype(np.float32)
    W4 = Wm.reshape(GI, BS, GJ, BS)  # block (i, j) = W4[i, :, j, :]
    strips = {}
    for r2 in range(2):
        if strip_cols[r2] == 0:
            continue
        tops, bots, JJ, U0, U1 = [], [], [], [], []
        for J in range(N_J):
            for C, p, u0, u1, j in plan[J]["cells"][r2]:
                tops.append(perms[C][2 * p])
                bots.append(perms[C][2 * p + 1])
                JJ.append(j)
                U0.append(u0)
                U1.append(u1)
        tops = np.asarray(tops)
        bots = np.asarray(bots)
        JJ = np.asarray(JJ)
        U0 = np.asarray(U0, bool)[:, None, None]
        U1 = np.asarray(U1, bool)[:, None, None]
        top = np.where(U0, W4[tops, :, JJ, :], np.float32(0))   # [n, 32, 32]
        bot = np.where(U1, W4[bots, :, JJ, :], np.float32(0))
        panel = np.concatenate([top, bot], axis=1)  # [n, 64, 32]
        strips[r2] = np.ascontiguousarray(
            panel.transpose(1, 0, 2).reshape(2 * BS, -1)
        ).astype(BF16)

    XG = 4
    xf = np.ascontiguousarray(x).reshape(B * S, IN_F)
    in_maps = []
    for c in range(N_CORES):
        xt0 = np.ascontiguousarray(
            xf[c * M_CORE : (c + 1) * M_CORE].T
        ).astype(BF16).reshape(GI, BS, M_CORE)
        groups = []
        for perm in perms:
            xt = xt0[perm].reshape(N_T, 128, M_CORE)
            # group layout: [NGC, 128, N_MSL, XG*MSL] — per (group, m),
            # the XG tiles' rows are contiguous in DRAM (4KB lines)
            groups.append(
                xt.reshape(N_T // XG, XG, 128, N_MSL, MSL)
                .transpose(0, 2, 3, 1, 4)
                .reshape(N_T // XG, 128, N_MSL, XG * MSL)
            )
        m = {"xt": np.ascontiguousarray(np.concatenate(groups, axis=0))}
        for r2, arr in strips.items():
            m[f"w{r2}"] = arr
        in_maps.append(m)

    # --- run -------------------------------------------------------------
    res = run_bass_kernel_spmd(nc, in_maps, list(range(N_CORES)), trace=True)

    # --- host-side output assembly --------------------------------------
    # device out row 128*J + 32*c + lane holds output column 32*jsel[J][c]+lane
    out_perm = np.empty(OUT_F, dtype=np.int64)
    lane = np.arange(BS)
    for J in range(N_J):
        for c in range(4):
            out_perm[128 * J + 32 * c + lane] = 32 * jsel[J][c] + lane
    y = np.empty((B * S, OUT_F), dtype=np.float32)
    for c in range(N_CORES):
        raw = res.results[c]["out"].astype(np.float32)  # [OUT_F(dev), M_CORE]
        y[c * M_CORE : (c + 1) * M_CORE, out_perm] = raw.T
    y = y.reshape(B, S, OUT_F)
    if np.any(bias):
        # bias is all-zero in this problem's setup; handled host-side for
        # generality.
        y = y + bias.astype(np.float32)
    kernel.last_exec_time_ns = res.exec_time_ns
    return y


# revision 55
# speedup vs baseline: 1.0517x; 1.0517x over previous
"""Block-sparse linear kernel for Trainium2 (8 NeuronCores, SPMD data-parallel).

Computes y = x @ (W * mask) + bias for
    x    [8, 1024, 4096] f32
    W    [4096, 4096]    f32
    mask [4096, 4096]    int32 (32x32-block structured, ~25% block density)
    bias [4096]          f32
    y    [8, 1024, 4096] f32

Strategy
--------
- Data parallel: core c computes rows [1024c, 1024(c+1)) of the flattened
  [8192, 4096] activation (i.e. batch element c).
- The trn2 PE array runs in 64x32 tiling mode (8 concurrent sub-array
  positions).  HW-measured: a tiled LDWEIGHTS+MATMUL stream sustains a
  FLAT ~31-32ns per MATMUL at 8 positions regardless of N (128/256/512),
  LDW count, or 16-way tiling — so the only lever that matters is the
  MATMUL COUNT.  The kernel therefore packs present 32x32 blocks into as
  few 64x32 panels as possible:
    * TWO copies of x live in SBUF under different block-row pairings
      (max-weight matching A; complementary matching B targeting the
      columns A leaves single).  Each block-column covers its present
      blocks by a per-column min edge cover over the union of the two
      pairings (panels per m-slice drop ~13%; every present block is
      covered exactly once, unused panel slots are zero-filled).
    * Block-columns are assigned to (supertile, col-slot) by a local
      search balancing the 8 per-supertile position queues (round-robin
      drain => imbalance = tail rounds with idle positions).
- Each weight panel is loaded into the PE array ONCE and used for BOTH
  512-column m-slices: bass's legalizer emits an LDWEIGHTS per matmul and
  a post-pass deletes the duplicate (the array keeps the stationary
  operand across matmuls — validated on HW).
- Concurrent DMA traffic measurably degrades the PE issue rate, so DMA is
  shaped: x tiles ride 4KB-line grouped DMAs (descriptor-rate-bound
  queues), copy A's m0 halves land first and a m0-only chunk-major ramp
  over copy-A-only supertiles tracks their arrival, weights prefetch
  W_PRE supertiles ahead (strip r2=0 on the SWDGE gpsimd queue), and the
  evacuation is one [128,1024] bf16 DMA per supertile.
- The two 64-row groups write disjoint PSUM banks; VectorE reduces the 2
  partial banks straight into a bf16 tile (halves evac DMA bytes; the
  harness gate is 2e-2 rel err, bf16 output adds ~2e-3).  Host assembles
  the final fp32 output, un-permuting the supertile column assignment.
"""

import numpy as np
import ml_dtypes

B, S, IN_F, OUT_F = 8, 1024, 4096, 4096
BS = 32                      # sparsity block size
GI, GJ = IN_F // BS, OUT_F // BS
GP = GI // 2                 # vertical super-rows (64 rows each)
N_CORES = 8
M_CORE = (B * S) // N_CORES  # rows of x per core (1024)
MSL = 512                    # m-slice width (one PSUM bank of fp32)
N_MSL = M_CORE // MSL        # 2
JCOLS = 4                    # output block-columns per supertile (4*32 = 128 partitions)
N_J = GJ // JCOLS            # 32 output supertiles
N_T = IN_F // 128            # 32 x tiles of 128 rows

BF16 = ml_dtypes.bfloat16


def _ensure_ntff_hook():
    """Best-effort: make trace=True work under axon when the image's antenv
    lacks axon_hooks.  Harmless if it fails — tracing is skipped, results
    are still correct."""
    import sys, types
    try:
        import antenv  # noqa
    except ImportError:
        return
    try:
        from antenv.axon_hooks import get_axon_ntff_profile_hook
        if get_axon_ntff_profile_hook() is not None:
            return
        mod = sys.modules["antenv.axon_hooks"]
    except ImportError:
        mod = types.ModuleType("antenv.axon_hooks")
        mod._hook = None
        def set_axon_ntff_profile_hook(h, _m=mod):
            _m._hook = h
        def get_axon_ntff_profile_hook(_m=mod):
            return _m._hook
        mod.set_axon_ntff_profile_hook = set_axon_ntff_profile_hook
        mod.get_axon_ntff_profile_hook = get_axon_ntff_profile_hook
        sys.modules["antenv.axon_hooks"] = mod
        import antenv as _a
        _a.axon_hooks = mod
    try:
        from trn_agent_boot.trn_boot import _ntff_profile_via_ctypes
        mod.set_axon_ntff_profile_hook(
            _ntff_profile_via_ctypes("/opt/axon/libaxon_pjrt.so")
        )
    except Exception:
        pass


def _pair_permutation(nzb):
    """Order block-rows so vertically-paired rows co-occur in many columns.

    Greedy max-weight matching on C[a,b] = #columns where blocks a and b are
    both present; each matched pair becomes one 64-row super-row, so high
    weight = fewer half-empty 64x32 panels = fewer matmuls.
    """
    C = nzb.astype(np.int32) @ nzb.astype(np.int32).T
    pairs = []
    try:
        import networkx as nx
        G = nx.Graph()
        for a in range(GI):
            for b in range(a + 1, GI):
                G.add_edge(a, b, weight=int(C[a, b]))
        pairs = [
            (int(min(a, b)), int(max(a, b)))
            for a, b in nx.max_weight_matching(G, maxcardinality=True)
        ]
    except Exception:
        pass
    if len(pairs) != GI // 2:
        pairs = []
        iu = np.triu_indices(GI, k=1)
        order = np.argsort(C[iu])[::-1]
        used = np.zeros(GI, dtype=bool)
        for idx in order:
            a, b = iu[0][idx], iu[1][idx]
            if not used[a] and not used[b]:
                used[a] = used[b] = True
                pairs.append((int(a), int(b)))
                if len(pairs) == GI // 2:
                    break
    perm = []
    for a, b in pairs:
        perm.extend((a, b))
    for a in range(GI):      # safety for odd leftovers
        if a not in perm:
            perm.append(a)
    return np.asarray(perm)


def _second_matching(nzb, permA):
    """Complementary pairing: match rows that copy A leaves as singletons in
    many columns."""
    partnerA = {}
    for p in range(GI // 2):
        a, b = int(permA[2 * p]), int(permA[2 * p + 1])
        partnerA[a] = b
        partnerA[b] = a
    presentA_partner = np.zeros((GI, GJ), bool)
    for r in range(GI):
        presentA_partner[r] = nzb[partnerA[r]]
    waste = nzb & ~presentA_partner
    Wb = waste.astype(np.int32) @ waste.astype(np.int32).T
    for a, b in partnerA.items():
        Wb[a, b] = -1000
    pairs = []
    try:
        import networkx as nx
        G = nx.Graph()
        for a in range(GI):
            for b in range(a + 1, GI):
                G.add_edge(a, b, weight=int(Wb[a, b]) + 1000)
        pairs = [(min(a, b), max(a, b))
                 for a, b in nx.max_weight_matching(G, maxcardinality=True)]
    except Exception:
        pass
    if len(pairs) != GI // 2:
        pairs = []
        iu = np.triu_indices(GI, k=1)
        order = np.argsort(Wb[iu])[::-1]
        used = np.zeros(GI, dtype=bool)
        for idx in order:
            a, b = int(iu[0][idx]), int(iu[1][idx])
            if not used[a] and not used[b]:
                used[a] = used[b] = True
                pairs.append((a, b))
                if len(pairs) == GI // 2:
                    break
    permB = []
    for a, b in pairs:
        permB.extend((a, b))
    for a in range(GI):
        if a not in permB:
            permB.append(a)
    return np.asarray(permB)


def _cover_columns(nzb, perms, a_only_cols):
    """Per column, cover the present rows with pair-panels from the given
    copies (min edge cover over the union of the pair matchings).

    nzb: [GI, GJ] presence in ORIGINAL row indices.
    perms: [permA, permB].
    a_only_cols: set of j restricted to copy A (their x copy-B data would
    not be resident early enough).

    Returns covers[j] = list of (copy, p, use0, use1) — pair index p of
    that copy; use0/use1 say whether the top/bottom 32-row slot carries
    real weights (False slots are zero-filled).  Every present block is
    covered exactly once.
    """
    pair_of = []       # per copy: row -> (p, slot)
    for perm in perms:
        m = {}
        for p in range(GI // 2):
            m[int(perm[2 * p])] = (p, 0)
            m[int(perm[2 * p + 1])] = (p, 1)
        pair_of.append(m)

    covers = []
    for j in range(GJ):
        R = set(np.where(nzb[:, j])[0].tolist())
        ncopies = 1 if j in a_only_cols else len(perms)
        # full edges: both endpoints present, edge = a copy's pair
        adj = {r: [] for r in R}
        for C in range(ncopies):
            perm = perms[C]
            for p in range(GI // 2):
                a, b = int(perm[2 * p]), int(perm[2 * p + 1])
                if a in R and b in R:
                    adj[a].append((b, C, p))
                    adj[b].append((a, C, p))
        matched = {}
        visited = set()
        for start in sorted(R):
            if start in visited:
                continue
            comp = set()
            stack = [start]
            while stack:
                v = stack.pop()
                if v in comp:
                    continue
                comp.add(v)
                for w, _C, _p in adj[v]:
                    if w not in comp:
                        stack.append(w)
            visited |= comp
            deg = {v: len(adj[v]) for v in comp}
            ends = sorted(v for v in comp if deg[v] <= 1)
            cur = ends[0] if ends else min(comp)
            prev = None
            order = []
            seen = set()
            while cur is not None and cur not in seen:
                order.append(cur)
                seen.add(cur)
                nxt = None
                for w, _C, _p in adj[cur]:
                    if w != prev and w not in seen:
                        nxt = w
                        break
                prev, cur = cur, nxt
            i = 0
            while i < len(order):
                if i + 1 < len(order):
                    e = next(((C, p) for w, C, p in adj[order[i]]
                              if w == order[i + 1]), None)
                    if e is not None:
                        matched[order[i]] = (e, order[i + 1])
                        matched[order[i + 1]] = (e, order[i])
                        i += 2
                        continue
                i += 1
        panels = {}
        for r in sorted(R):
            if r in matched:
                (C, p), partner = matched[r]
                slot = pair_of[C][r][1]
                key = (C, p)
                u = panels.setdefault(key, [False, False])
                u[slot] = True
            else:
                # singleton: use copy A's pair (or B if j allows and it
                # balances parity — keep simple: A)
                C = 0
                p, slot = pair_of[C][r]
                u = panels.setdefault((C, p + 10000 * (slot + 1)), [False, False])
                # distinct key per slot so two unrelated singletons that
                # happen to share a pair don't merge incorrectly — unless
                # they are the two slots of the same pair, which IS a valid
                # merge; handle below
                u[slot] = True
        # merge singleton keys back: (C, p+10000*(s+1)) pairs of same p
        merged = {}
        for (C, pk), u in panels.items():
            p = pk % 10000
            key = (C, p)
            mu = merged.setdefault(key, [False, False])
            mu[0] |= u[0]
            mu[1] |= u[1]
        covers.append([(C, p, u[0], u[1]) for (C, p), u in sorted(merged.items())])
    return covers


def _balance_jsel(covers, early_cols):
    """Assign block-columns j to (supertile J, col-slot c) so the 8 per-
    supertile position queues are balanced (the schedule drains round-robin,
    so spread = tail rounds with idle positions).  Columns in early_cols are
    pinned to the ramp supertiles (their covers use only copy A, whose x
    lands first).

    Returns jsel [N_J][4].
    """
    import random
    n = np.zeros((GJ, 2), np.int64)
    for j in range(GJ):
        for C, p, u0, u1 in covers[j]:
            n[j, p % 2] += 1

    def balance(cols, ngroups, iters):
        def loss(groups):
            tot = 0
            for g in groups:
                for r2 in range(2):
                    lens = [max(int(n[j, r2]), 1) for j in g]
                    tot += 4 * max(lens) - sum(lens)
            return tot

        cols = sorted(cols, key=lambda j: -int(n[j].sum()))
        groups = [cols[4 * J : 4 * J + 4] for J in range(ngroups)]
        cur = loss(groups)
        rng = random.Random(0)
        for _ in range(iters):
            a, b = rng.randrange(ngroups), rng.randrange(ngroups)
            if a == b:
                continue
            ia, ib = rng.randrange(4), rng.randrange(4)
            groups[a][ia], groups[b][ib] = groups[b][ib], groups[a][ia]
            nl = loss(groups)
            if nl <= cur:
                cur = nl
            else:
                groups[a][ia], groups[b][ib] = groups[b][ib], groups[a][ia]
        return groups

    n_early_groups = len(early_cols) // 4
    rest = [j for j in range(GJ) if j not in set(early_cols)]
    return (balance(list(early_cols), n_early_groups, 20000)
            + balance(rest, N_J - n_early_groups, 60000))


def _plan(covers, jsel):
    """Per-supertile weight storage layout and position queues (64x32
    pairing over two x copies).

    covers: per column j, list of (copy, p, use0, use1) pair-panels.
    jsel: [N_J][4] — block-column assigned to (J, c).

    Returns (plan, strip_cols):
      plan[J] = {
        'chunks': {r2: (src_col_base, n_cells)},     # weight DMA per strip
        'cells':  {r2: [(C, p, use0, use1, j), ...]},  # storage order
        'queues': {(r2, c): [(woff_or_None, C, p), ...]},
      }
    r2 = p % 2 (which 64-row half of the copy's x tile the pair occupies).
    woff None => dummy matmul with the zero-weight tile (position had no
    cells but its PSUM region must be initialized for the bank reduce).
    """
    plan = []
    strip_cols = [0, 0]
    for J in range(N_J):
        per_strip = {0: [], 1: []}
        ents = []
        for c in range(4):
            j = jsel[J][c]
            for C, p, u0, u1 in covers[j]:
                ents.append((C, p // 2, p, u0, u1, j, c))
        # x-arrival order: copy A's tiles land before copy B's
        ents.sort(key=lambda e: (e[0], e[1], e[6]))
        for C, lt, p, u0, u1, j, c in ents:
            per_strip[p % 2].append((C, p, u0, u1, j, c))
        chunks = {}
        cells = {}
        queues = {}
        for r2 in range(2):
            lst = per_strip[r2]
            chunks[r2] = (strip_cols[r2], len(lst))
            cells[r2] = [(C, p, u0, u1, j) for C, p, u0, u1, j, _c in lst]
            strip_cols[r2] += len(lst) * BS
            for k, (C, p, u0, u1, j, c) in enumerate(lst):
                queues.setdefault((r2, c), []).append((k * BS, C, p))
        for r2 in range(2):
            for c in range(4):
                if (r2, c) not in queues:
                    queues[(r2, c)] = [(None, 0, r2)]
        plan.append({"chunks": chunks, "cells": cells, "queues": queues})
    return plan, strip_cols


def _dedup_ldweights(nc):
    """Delete InstLdweights whose weights AP + tile position match the most
    recently loaded weights in the same basic block.  The PE array keeps the
    stationary operand across matmuls (validated on HW), so the reload is
    pure overhead on the weight-streaming path.
    """
    ndel = 0
    for f in nc.m.functions:
        for bb in f.blocks:
            insts = bb.instructions
            keep = []
            last = None
            for ins in insts:
                tn = type(ins).__name__
                if tn == 'InstLdweights':
                    k = (str(ins.ins[0]), str(ins.tile_position),
                         str(ins.tile_size), str(ins.perf_mode))
                    if k == last:
                        si = ins.sync_info
                        assert si is None or (
                            len(si.on_wait) == 0 and len(si.on_update) == 0
                        ), f"dup LDW {ins.name} carries sync info"
                        ndel += 1
                        continue
                    last = k
                elif tn == 'InstMatmult' and ins.is_transpose:
                    last = None  # transpose loads identity into the array
                keep.append(ins)
            if len(keep) != len(insts):
                while len(insts):
                    insts.pop()
                for i in keep:
                    insts.append(i)
    return ndel


def _build_program(plan, strip_cols):
    import concourse.bacc as bacc
    import concourse.tile as tile
    import concourse.mybir as mybir

    nc = bacc.Bacc(debug=False)
    bf16, f32 = mybir.dt.bfloat16, mybir.dt.float32

    XG = 4                       # x tiles per DMA group (4KB DRAM lines)
    N_G = (2 * N_T) // XG        # two x copies: A = groups 0..7, B = 8..15
    xt_d = nc.declare_dram_parameter(
        "xt", [N_G, 128, N_MSL, XG * MSL], bf16, isOutput=False
    )
    w_d = {}
    for r2 in range(2):
        if strip_cols[r2] > 0:
            w_d[r2] = nc.declare_dram_parameter(
                f"w{r2}", [2 * BS, strip_cols[r2]], bf16, isOutput=False
            )
    out_d = nc.declare_dram_parameter("out", [OUT_F, M_CORE], bf16, isOutput=True)

    # Largest per-(J, strip) weight chunk, in columns (>= BS for the tile alloc).
    lmax = BS
    for p in plan:
        for r2 in range(2):
            lmax = max(lmax, p["chunks"][r2][1] * BS)

    W_PRE = 8   # weight prefetch depth in supertiles
    N_GEN = 4   # ramp supertiles (m0-only pass, then m1 pass)

    with tile.TileContext(nc) as tc:
        with (
            tc.tile_pool(name="xp", bufs=1) as xp,
            tc.tile_pool(name="zp", bufs=1) as zp,
            tc.tile_pool(name="wp", bufs=W_PRE + 2) as wp,
            tc.tile_pool(name="ep", bufs=10) as ep,
            tc.tile_pool(name="pp", bufs=4, space="PSUM") as pp,
        ):
            # Queue assignment: x and evacs ride the HWDGE queues (sync,
            # scalar); weight strip r2=0 rides the SWDGE gpsimd queue (its
            # latency is hidden by the W_PRE prefetch depth), strip r2=1
            # alternates the HWDGE queues.
            def load_w(J):
                wt = wp.tile([128, lmax], bf16, tag="wt")
                for r2 in range(2):
                    base, ncell = plan[J]["chunks"][r2]
                    if ncell:
                        eng = (
                            nc.gpsimd if r2 == 0
                            else (nc.sync if J % 2 == 0 else nc.scalar)
                        )
                        eng.dma_start(
                            wt[64 * r2 : 64 * r2 + 64, : ncell * BS],
                            w_d[r2][:, base : base + ncell * BS],
                        )
                return wt

            Xg = {}

            def load_xg(g, m):
                # one DMA per (4-tile group, m-slice): 4KB contiguous per
                # partition on both the DRAM and SBUF side, so the
                # descriptor-rate-bound DMA queues move 4KB per descriptor;
                # m0 chunks land first — they are all the m0-only ramp needs
                if g not in Xg:
                    xg = xp.tile([128, N_MSL, XG, MSL], bf16, tag=f"xg{g}")
                    Xg[g] = xg
                (nc.sync if g % 2 == 0 else nc.scalar).dma_start(
                    Xg[g][:, m], xt_d[g][:, m]
                )

            # DMA emission order: ramp weights interleaved with copy A's m0
            # groups (the m0-only ramp input), then copy B m0 alternating
            # with copy A m1, then copy B m1; remaining weights are emitted
            # inside the J loop, W_PRE supertiles ahead.
            NGC = N_G // 2           # groups per copy
            wts = {0: load_w(0)}
            load_xg(0, 0)
            wts[1] = load_w(1)
            load_xg(1, 0)
            wts[2] = load_w(2)
            load_xg(2, 0)
            wts[3] = load_w(3)
            for g in range(3, NGC):
                load_xg(g, 0)
            wts[4] = load_w(4)
            for g in range(NGC):
                load_xg(NGC + g, 0)   # copy B m0
                load_xg(g, 1)         # copy A m1
                if g == 2:
                    wts[5] = load_w(5)
            for g in range(NGC):
                load_xg(NGC + g, 1)   # copy B m1
            zw = zp.tile([128, BS], bf16)
            nc.vector.memset(zw[:], 0.0)

            def emit_mm(P, wt, r2, c, woff, C, p, m, start, stop):
                lhsT = (
                    zw[64 * r2 : 64 * r2 + 64, :]
                    if woff is None
                    else wt[64 * r2 : 64 * r2 + 64, woff : woff + BS]
                )
                t = C * N_T + p // 2   # global x tile of this pair
                nc.tensor.matmul(
                    P[32 * c : 32 * c + 32, r2, :],
                    lhsT,
                    Xg[t // XG][64 * r2 : 64 * r2 + 64, m, t % XG, :],
                    start=start,
                    stop=stop,
                    tile_position=(64 * r2, 32 * c),
                )

            def emit_pair(P0, P1, wt, r2, c, woff, C, p, start, stop):
                emit_mm(P0, wt, r2, c, woff, C, p, 0, start, stop)
                emit_mm(P1, wt, r2, c, woff, C, p, 1, start, stop)

            def emit_reduce(P, ov, m):
                with nc.allow_low_precision(
                    reason="bf16 output; harness gate is 2e-2 rel err"
                ):
                    nc.vector.reduce_sum(
                        ov[:, m * MSL : (m + 1) * MSL],
                        P[:].transpose([0, 2, 1]),
                        axis=mybir.AxisListType.X,
                    )

            def emit_out(ov, J):
                # one [128, 1024] DMA per supertile: 2KB DRAM lines.  Early
                # supertiles evacuate on the SWDGE gpsimd queue so the HWDGE
                # queues stay clear for x during the landing-critical window;
                # evac latency is absorbed by the ep pool depth.
                eng = (
                    nc.gpsimd if J < 12
                    else (nc.scalar if J % 2 == 0 else nc.sync)
                )
                eng.dma_start(out_d[128 * J : 128 * (J + 1), :], ov[:])

            POS = [(r2, c) for r2 in range(2) for c in range(4)]

            # Ramp: the first N_GEN supertiles' queues merged chunk-major
            # (all ramp supertiles' panels for x tile t before any of tile
            # t+1) and m0-only first — early compute tracks x-group arrival;
            # the m1 pass over the same supertiles follows.  PSUM: N_GEN
            # single-m groups = all 8 banks.
            GEN0 = list(range(min(N_GEN, N_J)))
            merged = []
            for J in GEN0:
                for (r2, c), q in plan[J]["queues"].items():
                    for k, (woff, C, p) in enumerate(q):
                        t = -1 if woff is None else C * N_T + p // 2
                        merged.append((t, k, J, r2, c, woff, C, p))
            merged.sort(key=lambda e: (e[0], e[1], e[2]))
            first_of = {}
            last_of = {}
            for idx, e in enumerate(merged):
                key = (e[2], e[3], e[4])
                first_of.setdefault(key, idx)
                last_of[key] = idx

            OV = {}
            for J in GEN0:
                ovg = ep.tile([128, M_CORE], bf16, tag="ov")
                OV[J] = ovg
            for m in range(N_MSL):
                P_gen = {}
                for J in GEN0:
                    Pg = pp.tile([128, 2, MSL], f32, tag="P")
                    P_gen[J] = Pg
                for idx, (t, k, J, r2, c, woff, C, p) in enumerate(merged):
                    key = (J, r2, c)
                    emit_mm(
                        P_gen[J], wts[J], r2, c, woff, C, p, m,
                        first_of[key] == idx, last_of[key] == idx,
                    )
                for J in GEN0:
                    emit_reduce(P_gen[J], OV[J], m)
                    if m == N_MSL - 1:
                        emit_out(OV[J], J)

            # Main loop: globally pipelined round-robin over the 8 positions
            # with a 2-supertile PSUM window — a position that drains its
            # queue for supertile J immediately starts on J+1, so supertile
            # tails don't idle positions.
            PIPELINE = _build_program.pipeline
            if not PIPELINE:
                next_w = max(wts.keys()) + 1
                for J in range(len(GEN0), N_J):
                    while next_w < min(J + W_PRE, N_J):
                        wts[next_w] = load_w(next_w)
                        next_w += 1
                    wt = wts.pop(J)
                    P0 = pp.tile([128, 2, MSL], f32, tag="P")
                    P1 = pp.tile([128, 2, MSL], f32, tag="P")
                    qlists = [plan[J]["queues"][p] for p in POS]
                    idx = [0] * len(qlists)
                    remaining = sum(len(q) for q in qlists)
                    while remaining:
                        for qi, q in enumerate(qlists):
                            if idx[qi] < len(q):
                                woff, C, p = q[idx[qi]]
                                emit_pair(
                                    P0, P1, wt, POS[qi][0], POS[qi][1],
                                    woff, C, p,
                                    idx[qi] == 0, idx[qi] == len(q) - 1,
                                )
                                idx[qi] += 1
                                remaining -= 1
                    ov = ep.tile([128, M_CORE], bf16, tag="ov")
                    emit_reduce(P0, ov, 0)
                    emit_reduce(P1, ov, 1)
                    emit_out(ov, J)
            else:
                next_w = max(wts.keys()) + 1
                live = []
                pos_at = {}
                qpos = {}          # (J, p) -> next entry index

                def open_J(J):
                    nonlocal next_w
                    while next_w < min(J + W_PRE, N_J):
                        wts[next_w] = load_w(next_w)
                        next_w += 1
                    P0 = pp.tile([128, 2, MSL], f32, tag="P")
                    P1 = pp.tile([128, 2, MSL], f32, tag="P")
                    live.append({"J": J, "P": (P0, P1), "wt": wts.pop(J),
                                 "done": set()})

                def close_J(entry):
                    J = entry["J"]
                    ov = ep.tile([128, M_CORE], bf16, tag="ov")
                    emit_reduce(entry["P"][0], ov, 0)
                    emit_reduce(entry["P"][1], ov, 1)
                    emit_out(ov, J)

                J0 = len(GEN0)
                for p in POS:
                    pos_at[p] = J0
                open_J(J0)
                if J0 + 1 < N_J:
                    open_J(J0 + 1)
                while live:
                    progressed = False
                    for p in POS:
                        Jp = pos_at[p]
                        entry = next((e for e in live if e["J"] == Jp), None)
                        if entry is None:
                            continue
                        q = plan[Jp]["queues"][p]
                        k = qpos.get((Jp, p), 0)
                        if k >= len(q):
                            entry["done"].add(p)
                            if any(e["J"] == Jp + 1 for e in live):
                                pos_at[p] = Jp + 1
                            continue
                        woff, C, pp_ = q[k]
                        emit_pair(
                            entry["P"][0], entry["P"][1], entry["wt"],
                            p[0], p[1], woff, C, pp_,
                            k == 0, k == len(q) - 1,
                        )
                        qpos[(Jp, p)] = k + 1
                        progressed = True
                    head = live[0]
                    if len(head["done"]) == len(POS):
                        close_J(head)
                        live.pop(0)
                        nxt = (live[-1]["J"] + 1) if live else head["J"] + 1
                        if nxt < N_J:
                            open_J(nxt)
                            for p in POS:
                                if pos_at[p] < live[0]["J"]:
                                    pos_at[p] = live[0]["J"]
                    if not progressed:
                        for e in live:
                            for p in POS:
                                if p not in e["done"]:
                                    q = plan[e["J"]]["queues"][p]
                                    if qpos.get((e["J"], p), 0) >= len(q):
                                        e["done"].add(p)

    ndel = _dedup_ldweights(nc)
    nc.compile()
    nc._ldw_dedup_count = ndel
    return nc


_build_program.pipeline = False


_CACHE = {}


def kernel(x, W, bias, mask):
    assert x.shape == (B, S, IN_F) and W.shape == (IN_F, OUT_F)
    _ensure_ntff_hook()
    from concourse.bass_utils import run_bass_kernel_spmd

    # --- host-side input prep -------------------------------------------
    mask_nz = mask != 0
    nzb = np.asarray(mask_nz.reshape(GI, BS, GJ, BS).any(axis=(1, 3)))

    key = nzb.tobytes()
    if key not in _CACHE:
        # alternating optimization of the two pairings: re-complement each
        # against the other and keep the pair with the smallest total cover
        permA = _pair_permutation(nzb)
        permB = _second_matching(nzb, permA)
        best = None
        cand = (permA, permB)
        for _ in range(3):
            size = sum(len(c) for c in _cover_columns(nzb, list(cand), set()))
            if best is None or size < best[0]:
                best = (size, cand)
            cand = (_second_matching(nzb, cand[1]), cand[1])
            size = sum(len(c) for c in _cover_columns(nzb, list(cand), set()))
            if size < best[0]:
                best = (size, cand)
            cand = (cand[0], _second_matching(nzb, cand[0]))
        perms = list(best[1])
        # pick ramp (copy-A-only) columns: those that gain least from copy B
        covA = _cover_columns(nzb, perms, set(range(GJ)))
        covAB = _cover_columns(nzb, perms, set())
        penalty = [len(covA[j]) - len(covAB[j]) for j in range(GJ)]
        N_EARLY = 32
        early = sorted(range(GJ), key=lambda j: (penalty[j], j))[:N_EARLY]
        covers = _cover_columns(nzb, perms, set(early))
        jsel = _balance_jsel(covers, early)
        plan, strip_cols = _plan(covers, jsel)
        nc = _build_program(plan, strip_cols)
        _CACHE[key] = (perms, jsel, plan, strip_cols, nc)
    perms, jsel, plan, strip_cols, nc = _CACHE[key]

    # Masked weights, gathered per row strip in plan storage order.  Unused
    # panel slots (use flag False) are zero-filled: their block may be
    # present in the mask but is covered by another panel.
    Wm = np.where(mask_nz, W, np.float32(0)).astype(np.float32)
    W4 = Wm.reshape(GI, BS, GJ, BS)  # block (i, j) = W4[i, :, j, :]
    strips = {}
    for r2 in range(2):
        if strip_cols[r2] == 0:
            continue
        tops, bots, JJ, U0, U1 = [], [], [], [], []
        for J in range(N_J):
            for C, p, u0, u1, j in plan[J]["cells"][r2]:
                tops.append(perms[C][2 * p])
                bots.append(perms[C][2 * p + 1])
                JJ.append(j)
                U0.append(u0)
                U1.append(u1)
        tops = np.asarray(tops)
        bots = np.asarray(bots)
        JJ = np.asarray(JJ)
        U0 = np.asarray(U0, bool)[:, None, None]
        U1 = np.asarray(U1, bool)[:, None, None]
        top = np.where(U0, W4[tops, :, JJ, :], np.float32(0))   # [n, 32, 32]
        bot = np.where(U1, W4[bots, :, JJ, :], np.float32(0))
        panel = np.concatenate([top, bot], axis=1)  # [n, 64, 32]
        strips[r2] = np.ascontiguousarray(
            panel.transpose(1, 0, 2).reshape(2 * BS, -1)
        ).astype(BF16)

    XG = 4
    xf = np.ascontiguousarray(x).reshape(B * S, IN_F)
    in_maps = []
    for c in range(N_CORES):
        xt0 = np.ascontiguousarray(
            xf[c * M_CORE : (c + 1) * M_CORE].T
        ).astype(BF16).reshape(GI, BS, M_CORE)
        groups = []
        for perm in perms:
            xt = xt0[perm].reshape(N_T, 128, M_CORE)
            # group layout: [NGC, 128, N_MSL, XG*MSL] — per (group, m),
            # the XG tiles' rows are contiguous in DRAM (4KB lines)
            groups.append(
                xt.reshape(N_T // XG, XG, 128, N_MSL, MSL)
                .transpose(0, 2, 3, 1, 4)
                .reshape(N_T // XG, 128, N_MSL, XG * MSL)
            )
        m = {"xt": np.ascontiguousarray(np.concatenate(groups, axis=0))}
        for r2, arr in strips.items():
            m[f"w{r2}"] = arr
        in_maps.append(m)

    # --- run -------------------------------------------------------------
    res = run_bass_kernel_spmd(nc, in_maps, list(range(N_CORES)), trace=True)

    # --- host-side output assembly --------------------------------------
    # device out row 128*J + 32*c + lane holds output column 32*jsel[J][c]+lane
    out_perm = np.empty(OUT_F, dtype=np.int64)
    lane = np.arange(BS)
    for J in range(N_J):
        for c in range(4):
            out_perm[128 * J + 32 * c + lane] = 32 * jsel[J][c] + lane
    y = np.empty((B * S, OUT_F), dtype=np.float32)
    for c in range(N_CORES):
        raw = res.results[c]["out"].astype(np.float32)  # [OUT_F(dev), M_CORE]
        y[c * M_CORE : (c + 1) * M_CORE, out_perm] = raw.T
    y = y.reshape(B, S, OUT_F)
    if np.any(bias):
        # bias is all-zero in this problem's setup; handled host-side for
        # generality.
        y = y + bias.astype(np.float32)
    kernel.last_exec_time_ns = res.exec_time_ns
    return y
